# revision 16
# baseline (speedup 1.0000x reference)
"""Trainium2 Bass kernel for nn_NodeLevelAttentionImproved (GAT-style layer).

Math (see reference):
  h_proj = h @ W                              [N, 256]
  el/er  = per-head dots of h_proj with a_l/a_r   [N, 4]
  e[n,m,h]   = leaky_relu(el[n,h] + er[idx[n,m],h], 0.2), masked -> softmax over m
  out_heads  = sum_m alpha * h_heads[idx]     [N, 4, 64]
  out = LayerNorm(gelu_erf(out_heads.flat + h_proj)) * gamma + beta

Strategy (8 cores, no collectives — each core recomputes the full projection):
  Host: nodes are sorted by unmasked-neighbor count and dealt, tile-by-tile,
  round-robin to the 8 cores, so all cores share one per-round gather-slot
  schedule K_sched (identical NEFF on every core) while skipping masked
  neighbor slots entirely (~2x fewer gather rows than the dense M=32 layout).

  phase 1: full h_proj via PE (fp16), streamed to a DRAM table of bare
           512-byte feature rows (sorted node order).
  phase 2: per output tile of 128 nodes, chunked dma_gather of K_t
           unmasked-neighbor rows + the self row. The per-head feature
           blocks are pre-rotated (host-side orthogonal basis) so that
           component 0 IS er and el = c0*f0 + c1*f1 of the self row -- no
           DVE dot products. Masked/padded slots gather a sentinel row
           whose er component is -150 (dies in the softmax exp; no mask
           tensor, no max subtraction; exp in fp32). Alpha is applied by
           DVE broadcast-multiply; the slot-sum runs transposed on the PE
           (psum[f', n], residual folded in) followed by an un-rotation
           matmul back to psum[n, f]. Gelu + LayerNorm run in two batches
           (first one overlapped under the remaining gathers).
"""

import sys

for _p in ("/opt/trn_rl_repo", "/root/.axon_site/_ro/trn_rl_repo"):
    if _p not in sys.path:
        sys.path.insert(0, _p)

import numpy as np

import concourse.bacc as bacc
import concourse.bass as bass
import concourse.mybir as mybir
import concourse.tile as tile
from concourse import library_config
from concourse.bass_utils import run_bass_kernel_spmd

F32 = mybir.dt.float32
F16 = mybir.dt.float16
I16 = mybir.dt.int16
AF = mybir.ActivationFunctionType
ALU = mybir.AluOpType
AX = mybir.AxisListType

# Problem constants (hardcoded per the harness contract).
N = 20000
M = 32          # neighbors
DIN = 256
DOUT = 256
H = 4
D = 64
LN_EPS = 1e-5
NCORES = 8
N_PAD = 20480
SHARD = N_PAD // NCORES
TILES = SHARD // 128          # 20 tiles of 128 nodes per core
GTILES = N_PAD // 128         # 160 global tiles

ROWE = 256       # fp16 elements per table row (512B)
KBLK = 2048      # h_T strip width for phase-1 loads
KCH = 7          # gather slots per chunk (7*128 = 896 descriptors)


def build_graph(nc, k_sched, n_strip=KBLK):
    """Emit the full per-core program into `nc` (inside a TileContext).

    k_sched: per-round gather-slot counts (neighbors only; the self row is
    always appended as slot K), shared by all cores.
    """
    assert len(k_sched) == TILES
    kpref = np.concatenate([[0], np.cumsum([k + 1 for k in k_sched])])
    tot_slots = int(kpref[-1])       # sum of (K_t + 1) over tiles
    kg_max = max(k_sched) + 1

    # ---- I/O ----
    hT = nc.dram_tensor("ht", [2 * 128, N_PAD], F16, kind="ExternalInput")
    wa = nc.dram_tensor("wa", [2 * 128, DOUT], F16, kind="ExternalInput")
    ident = nc.dram_tensor("ident", [128, 128], F16, kind="ExternalInput")
    unrot = nc.dram_tensor("unrot", [2 * 128, DOUT], F16, kind="ExternalInput")
    crep_d = nc.dram_tensor("crep", [128, 2 * H], F16, kind="ExternalInput")
    idx_d = nc.dram_tensor("idx", [128, tot_slots * 8], I16, kind="ExternalInput")
    sent_d = nc.dram_tensor("sent", [128, ROWE], F16, kind="ExternalInput")
    out_d = nc.dram_tensor("out", [SHARD, DOUT], F32, kind="ExternalOutput")

    with tile.TileContext(nc) as tc:
        import contextlib

        ctx = contextlib.ExitStack()
        with ctx:
            consts = ctx.enter_context(tc.tile_pool(name="consts", bufs=1))
            dram = ctx.enter_context(tc.tile_pool(name="dram", bufs=1, space="DRAM"))

            table = dram.tile([N_PAD + 128, ROWE], F16)

            # constants in
            wa0 = consts.tile([128, DOUT], F16)
            wa1 = consts.tile([128, DOUT], F16)
            nc.scalar.dma_start(out=wa0[:], in_=wa[0:128, :])
            nc.scalar.dma_start(out=wa1[:], in_=wa[128:256, :])
            idn = consts.tile([128, 128], F16)
            nc.scalar.dma_start(out=idn[:], in_=ident[:, :])
            # un-rotation matrix chunks + el-combination coefficients
            u0 = consts.tile([128, DOUT], F16)
            u1 = consts.tile([128, DOUT], F16)
            nc.scalar.dma_start(out=u0[:], in_=unrot[0:128, :])
            nc.scalar.dma_start(out=u1[:], in_=unrot[128:256, :])
            crep = consts.tile([128, 2 * H], F16)
            nc.scalar.dma_start(out=crep[:], in_=crep_d[:, :])
            idx_sb = consts.tile([128, tot_slots * 8], I16)
            nc.scalar.dma_start(out=idx_sb[:], in_=idx_d[:, :])
            # sentinel rows: features chosen so er_h = -150 for every head;
            # masked/padded slots gather them and die in the softmax exp.
            sent_sb = consts.tile([128, ROWE], F16)
            nc.scalar.dma_start(out=sent_sb[:], in_=sent_d[:, :])
            nc.sync.dma_start(out=table[N_PAD:N_PAD + 128, :], in_=sent_sb[:])

            nc.gpsimd.load_library(library_config.mlp)

            # ---------------- phase 1: projection + table build ----------------
            with (
                tc.tile_pool(name="strips", bufs=2) as strips,
                tc.tile_pool(name="p1psum", bufs=8, space="PSUM") as p1psum,
                tc.tile_pool(name="tab", bufs=4) as tabp,
            ):
                blk_per_strip = n_strip // 128
                for s in range(N_PAD // n_strip):
                    st0 = strips.tile([128, n_strip], F16, tag="st0")
                    st1 = strips.tile([128, n_strip], F16, tag="st1")
                    c0 = s * n_strip
                    nc.sync.dma_start(out=st0[:], in_=hT[0:128, c0:c0 + n_strip])
                    nc.sync.dma_start(out=st1[:], in_=hT[128:256, c0:c0 + n_strip])
                    for b in range(blk_per_strip):
                        g = s * blk_per_strip + b
                        ps = p1psum.tile([128, DOUT], F32)
                        nc.tensor.matmul(
                            out=ps[:],
                            lhsT=st0[:, b * 128:(b + 1) * 128],
                            rhs=wa0[:],
                            start=True, stop=False,
                        )
                        nc.tensor.matmul(
                            out=ps[:],
                            lhsT=st1[:, b * 128:(b + 1) * 128],
                            rhs=wa1[:],
                            start=False, stop=True,
                        )
                        tb = tabp.tile([128, ROWE], F16)
                        nc.scalar.copy(tb[:], ps[:])
                        nc.sync.dma_start(
                            out=table[g * 128:(g + 1) * 128, :], in_=tb[:]
                        )

            # ---------------- phase 2: gather / attention ----------------
            pre = consts.tile([128, TILES, DOUT], F32)   # pre-activation rows
            gbuf = consts.tile([128, TILES, DOUT], F32)
            vinv = consts.tile([128, TILES], F32)
            rstd = consts.tile([128, TILES], F32)
            mus = consts.tile([128, TILES], F32)

            def epilogue(ln, t0, t1):
                # gelu + LayerNorm (gamma=1, beta=0 per spec fills) + out DMA
                nc.scalar.activation(
                    gbuf[:, t0:t1, :].rearrange("p t f -> p (t f)"),
                    pre[:, t0:t1, :].rearrange("p t f -> p (t f)"),
                    AF.Gelu,
                )
                for t in range(t0, t1):
                    stats = ln.tile([128, 6], F32, tag="st")
                    nc.vector.bn_stats(out=stats[:], in_=gbuf[:, t, :])
                    mv = ln.tile([128, 2], F32, tag="mv")
                    nc.vector.bn_aggr(out=mv[:], in_=stats[:])
                    nc.vector.tensor_copy(mus[:, t:t + 1], mv[:, 0:1])
                    veps = ln.tile([128, 1], F32, tag="veps")
                    nc.vector.tensor_scalar_add(veps[:], mv[:, 1:2], LN_EPS)
                    nc.vector.reciprocal(vinv[:, t:t + 1], veps[:])
                nc.scalar.sqrt(rstd[:, t0:t1], vinv[:, t0:t1])
                for t in range(t0, t1):
                    nc.vector.scalar_tensor_tensor(
                        out=pre[:, t, :],
                        in0=gbuf[:, t, :],
                        scalar=mus[:, t:t + 1],
                        in1=rstd[:, t:t + 1].to_broadcast([128, DOUT]),
                        op0=ALU.subtract, op1=ALU.mult,
                    )
                nc.sync.dma_start(
                    out=out_d[t0 * 128:t1 * 128, :].rearrange(
                        "(t p) f -> p t f", p=128
                    ),
                    in_=pre[:, t0:t1, :],
                )

            with (
                tc.tile_pool(name="gat", bufs=3) as gat,
                tc.tile_pool(name="sc", bufs=3) as sc,
                tc.tile_pool(name="prod", bufs=2) as prodp,
                tc.tile_pool(name="ln", bufs=4) as lnp,
                tc.tile_pool(name="pT", bufs=2, space="PSUM") as pTp,
                tc.tile_pool(name="p2psum", bufs=2, space="PSUM") as p2psum,
            ):
                for t in range(TILES):
                    if t == TILES - 4:
                        epilogue(lnp, 0, TILES - 4)
                    K = k_sched[t]
                    KG = K + 1             # + self slot
                    s0 = int(kpref[t])     # slot offset into idx/mask arrays
                    G = gat.tile([128, kg_max, ROWE], F16, tag="G")
                    for c0 in range(0, KG, KCH):
                        c1 = min(c0 + KCH, KG)
                        ni = (c1 - c0) * 128
                        nc.gpsimd.dma_gather(
                            G[:, c0:c1, :],
                            table[:, :],
                            idx_sb[:, (s0 + c0) * 8:(s0 + c1) * 8],
                            ni,
                            ni,
                            ROWE,
                            elem_step=ROWE,
                        )

                    # rotated basis: er = feature 0 of each head block;
                    # el = c0*f0 + c1*f1 of the self row
                    elp = sc.tile([128, H, 2], F32, tag="elp")
                    selfv = G[:, K, :].rearrange("p (j d) -> p j d", d=D)[:, :, 0:2]
                    nc.vector.tensor_mul(
                        elp[:], selfv,
                        crep[:].rearrange("p (j c) -> p j c", c=2),
                    )
                    el = sc.tile([128, H], F32, tag="el")
                    nc.vector.tensor_reduce(
                        out=el[:], in_=elp[:], axis=AX.X, op=ALU.add
                    )
                    er_v = (
                        G[:, 0:K, :]
                        .rearrange("p k (j d) -> p j k d", d=D)[:, :, :, 0:1]
                        .rearrange("p j k o -> p j (k o)")
                    )
                    S = sc.tile([128, H, K], F32, tag="S")
                    nc.vector.tensor_add(
                        S[:], er_v, el[:, :, None].to_broadcast([128, H, K])
                    )
                    # leaky relu: (S*0.2) max S, then + maskln
                    S2 = sc.tile([128, H, K], F32, tag="S2")
                    nc.vector.scalar_tensor_tensor(
                        out=S2[:], in0=S[:], scalar=0.2, in1=S[:],
                        op0=ALU.mult, op1=ALU.max,
                    )
                    E = sc.tile([128, H, K], F32, tag="E")
                    nc.scalar.activation(E[:], S2[:], AF.Exp)
                    dsum = sc.tile([128, H], F32, tag="dsum")
                    nc.vector.tensor_reduce(
                        out=dsum[:], in_=E[:], axis=AX.X, op=ALU.add
                    )
                    rinv = sc.tile([128, H], F32, tag="rinv")
                    nc.vector.reciprocal(rinv[:], dsum[:])
                    alph = sc.tile([128, H, K], F16, tag="alph")
                    nc.vector.tensor_mul(
                        alph[:], E[:], rinv[:, :, None].to_broadcast([128, H, K])
                    )
                    # weighted neighbor features: alpha broadcast over d
                    prod = prodp.tile([128, kg_max, DOUT], F16, tag="prod")
                    al_v = alph[:].rearrange("p j k -> p k j")[:, :, :, None]
                    nc.vector.tensor_mul(
                        prod[:, 0:K, :],
                        G[:, 0:K, :].rearrange("p k (j d) -> p k j d", d=D),
                        al_v.to_broadcast([128, K, H, D]),
                    )
                    # transposed slot-sum on PE (psum[f', n]), then un-rotate
                    sbT0 = sc.tile([128, 128], F16, tag="sbT0")
                    sbT1 = sc.tile([128, 128], F16, tag="sbT1")
                    for c, sbT in ((0, sbT0), (1, sbT1)):
                        poT = pTp.tile([128, 128], F32, tag=f"poT{c}")
                        nc.tensor.matmul(
                            out=poT[:], lhsT=G[:, K, c * 128:(c + 1) * 128],
                            rhs=idn[:], start=True, stop=False,
                        )
                        for j in range(K):
                            nc.tensor.matmul(
                                out=poT[:],
                                lhsT=prod[:, j, c * 128:(c + 1) * 128],
                                rhs=idn[:], start=False, stop=(j == K - 1),
                            )
                        nc.vector.tensor_copy(sbT[:], poT[:])
                    p2t = p2psum.tile([128, DOUT], F32)
                    nc.tensor.matmul(
                        out=p2t[:], lhsT=sbT0[:], rhs=u0[:],
                        start=True, stop=False,
                    )
                    nc.tensor.matmul(
                        out=p2t[:], lhsT=sbT1[:], rhs=u1[:],
                        start=False, stop=True,
                    )
                    nc.vector.tensor_copy(pre[:, t, :], p2t[:])

                epilogue(lnp, TILES - 4, TILES)
    return nc


def build_nc(k_sched):
    nc = bacc.Bacc("TRN2", target_bir_lowering=False, debug=False)
    build_graph(nc, k_sched)
    nc.compile()
    return nc


# ---------------------------------------------------------------------------
# host-side planning + marshaling (pure layout/dtype work)
# ---------------------------------------------------------------------------

def plan(neighbor_mask):
    """Sort nodes by unmasked-degree into 160 tiles; deal tiles round-robin
    to cores; derive the shared per-round slot schedule."""
    cnt = np.zeros(N_PAD, np.int64)
    cnt[:N] = (neighbor_mask != 0).sum(1)
    order = np.argsort(-cnt, kind="stable")         # descending degree
    gtiles = order.reshape(GTILES, 128)             # global tile g, partition p
    tile_max = cnt[gtiles].max(1)                   # per-tile max degree
    k_sched = tuple(
        int(max(1, tile_max[8 * r:8 * r + 8].max())) for r in range(TILES)
    )
    rank = np.empty(N_PAD, np.int64)
    rank[order] = np.arange(N_PAD)                  # node -> sorted position
    return order, gtiles, rank, k_sched


def make_inputs(h, neighbor_idx, neighbor_mask, W, a_l, a_r,
                order, gtiles, rank, k_sched):
    kpref = np.concatenate([[0], np.cumsum([k + 1 for k in k_sched])])
    tot_slots = int(kpref[-1])

    # h columns in sorted order; pad nodes (id >= N) stay zero
    h16 = h.astype(np.float16)
    hT = np.zeros((2 * 128, N_PAD), np.float16)
    real_cols = np.where(order < N)[0]
    hT[:, real_cols] = h16[order[real_cols]].T

    ident = np.eye(128, dtype=np.float16)

    # per-head rotation: component 0 of each rotated block IS er, and
    # el = c0*f0 + c1*f1; un-rotation U maps the slot-sum back.
    Wr = np.zeros((DIN, DOUT), np.float64)
    Ufull = np.zeros((DOUT, DOUT), np.float64)
    cvec = np.zeros(2 * H, np.float64)
    W64 = W.astype(np.float64)
    for hh in range(H):
        ar = a_r[hh].astype(np.float64)
        al = a_l[hh].astype(np.float64)
        d0 = np.linalg.norm(ar)
        r0 = ar / d0
        v = al - (al @ r0) * r0
        nv = np.linalg.norm(v)
        if nv < 1e-9:
            v = np.zeros(D)
            v[int(np.argmin(np.abs(r0)))] = 1.0
            v -= (v @ r0) * r0
            nv = np.linalg.norm(v)
        r1 = v / nv
        q, _ = np.linalg.qr(np.column_stack([r0, r1, np.eye(D)]))
        R = q[:, :D].T
        if R[0] @ r0 < 0:
            R[0] = -R[0]
        if R[1] @ r1 < 0:
            R[1] = -R[1]
        Dv = np.ones(D)
        Dv[0] = d0
        M = R * Dv[:, None]
        U = R / Dv[:, None]
        blk = slice(hh * D, (hh + 1) * D)
        Wr[:, blk] = W64[:, blk] @ M.T
        Ufull[blk, blk] = U
        cvec[2 * hh] = (al @ r0) / d0
        cvec[2 * hh + 1] = al @ r1
    wa = np.ascontiguousarray(Wr.astype(np.float16))
    unrot = np.ascontiguousarray(Ufull.astype(np.float16))
    crep = np.ascontiguousarray(
        np.tile(cvec.astype(np.float16).reshape(1, 2 * H), (128, 1))
    )

    # sentinel features (rotated basis): er component = -150, rest 0
    sent_row = np.zeros(DOUT, np.float32)
    for hh in range(H):
        sent_row[hh * D] = -150.0
    sent = np.tile(sent_row.astype(np.float16), (128, 1))

    # per-node compacted neighbor lists in sorted-table coordinates
    nbr_rank = rank[neighbor_idx].astype(np.int16)  # [N, M]
    valid = neighbor_mask != 0
    vorder = np.argsort(~valid, axis=1, kind="stable")
    compacted = np.take_along_axis(nbr_rank, vorder, axis=1)  # valid prefix
    cnt = valid.sum(1).astype(np.int64)

    in_maps = []
    for c in range(NCORES):
        idx16 = np.full((tot_slots, 128), np.int16(N_PAD), np.int16)
        for r in range(TILES):
            K = k_sched[r]
            s0 = int(kpref[r])
            nodes = gtiles[8 * r + c]               # original node ids
            real = nodes < N
            nd = np.where(real, nodes, 0)
            sl = compacted[nd, :K].T                # [K, 128]
            have = (np.arange(K)[:, None] < cnt[nd][None, :]) & real[None, :]
            idx16[s0:s0 + K] = np.where(have, sl, np.int16(N_PAD))
            idx16[s0 + K] = ((8 * r + c) * 128
                             + np.arange(128)).astype(np.int16)  # self
        # wrap each slot-column group into 16 partitions, replicate x8
        flat = idx16.reshape(tot_slots * 128)
        wrapped = flat.reshape(tot_slots * 8, 16).T   # [16, tot*8]
        idx_in = np.ascontiguousarray(np.tile(wrapped, (8, 1)))
        in_maps.append({
            "ht": hT, "wa": wa, "ident": ident, "unrot": unrot,
            "crep": crep, "idx": idx_in, "sent": sent,
        })
    return in_maps


_CACHE = {}


def _get_nc(k_sched):
    if k_sched not in _CACHE:
        _CACHE[k_sched] = build_nc(k_sched)
    return _CACHE[k_sched]


def kernel(h, neighbor_idx, neighbor_mask, W, a_l, a_r, ln_gamma, ln_beta,
           **extra):
    assert h.shape[0] == N
    assert np.allclose(ln_gamma, 1.0) and np.allclose(ln_beta, 0.0), \
        "kernel assumes unit gamma / zero beta (per problem spec fills)"

    order, gtiles, rank, k_sched = plan(neighbor_mask)
    nc = _get_nc(k_sched)
    in_maps = make_inputs(
        h, neighbor_idx, neighbor_mask, W, a_l, a_r,
        order, gtiles, rank, k_sched,
    )
    res = run_bass_kernel_spmd(nc, in_maps, core_ids=list(range(NCORES)))
    # core c's local tile r = global tile 8r+c; sorted position = tile-major
    allout = np.stack(
        [res.results[c]["out"].reshape(TILES, 128, DOUT) for c in range(NCORES)]
    )                                              # [c, r, p, f]
    sorted_rows = allout.transpose(1, 0, 2, 3).reshape(N_PAD, DOUT)
    out = np.empty((N, DOUT), np.float32)
    sel = order < N
    out[order[sel]] = sorted_rows[sel]
    return out


# revision 17
# speedup vs baseline: 1.1786x; 1.1786x over previous
"""Trainium2 Bass kernel for nn_NodeLevelAttentionImproved (GAT-style layer).

Math (see reference):
  h_proj = h @ W                              [N, 256]
  el/er  = per-head dots of h_proj with a_l/a_r   [N, 4]
  e[n,m,h]   = leaky_relu(el[n,h] + er[idx[n,m],h], 0.2), masked -> softmax over m
  out_heads  = sum_m alpha * h_heads[idx]     [N, 4, 64]
  out = LayerNorm(gelu_erf(out_heads.flat + h_proj)) * gamma + beta

Strategy (8 cores, no collectives — each core recomputes the full projection):
  Host: nodes are sorted by unmasked-neighbor count and dealt, tile-by-tile,
  round-robin to the 8 cores, so all cores share one per-round gather-slot
  schedule K_sched (identical NEFF on every core) while skipping masked
  neighbor slots entirely (~2x fewer gather rows than the dense M=32 layout).

  phase 1: full h_proj via PE (fp16), streamed to a DRAM table of bare
           512-byte feature rows (sorted node order).
  phase 2: per output tile of 128 nodes, chunked dma_gather of K_t
           unmasked-neighbor rows + the self row. The per-head feature
           blocks are pre-rotated (host-side orthogonal basis) so that
           component 0 IS er and el = c0*f0 + c1*f1 of the self row -- no
           DVE dot products. Masked/padded slots gather a sentinel row
           whose er component is -150 (dies in the softmax exp; no mask
           tensor, no max subtraction; exp in fp32). Alpha is applied by
           DVE broadcast-multiply; the slot-sum runs transposed on the PE
           (psum[f', n], residual folded in) followed by an un-rotation
           matmul back to psum[n, f]. Gelu + LayerNorm run in two batches
           (first one overlapped under the remaining gathers).
"""

import sys

for _p in ("/opt/trn_rl_repo", "/root/.axon_site/_ro/trn_rl_repo"):
    if _p not in sys.path:
        sys.path.insert(0, _p)

import numpy as np

import concourse.bacc as bacc
import concourse.bass as bass
import concourse.mybir as mybir
import concourse.tile as tile
from concourse import library_config
from concourse.bass_utils import run_bass_kernel_spmd

F32 = mybir.dt.float32
F16 = mybir.dt.float16
I16 = mybir.dt.int16
AF = mybir.ActivationFunctionType
ALU = mybir.AluOpType
AX = mybir.AxisListType

# Problem constants (hardcoded per the harness contract).
N = 20000
M = 32          # neighbors
DIN = 256
DOUT = 256
H = 4
D = 64
LN_EPS = 1e-5
NCORES = 8
N_PAD = 20480
SHARD = N_PAD // NCORES
TILES = SHARD // 128          # 20 tiles of 128 nodes per core
GTILES = N_PAD // 128         # 160 global tiles

ROWE = 256       # fp16 elements per table row (512B)
KBLK = 2048      # h_T strip width for phase-1 loads
KCH = 7          # gather slots per chunk (7*128 = 896 descriptors)


def build_graph(nc, k_sched, n_strip=KBLK):
    """Emit the full per-core program into `nc` (inside a TileContext).

    k_sched: per-round gather-slot counts (neighbors only; the self row is
    always appended as slot K), shared by all cores.
    """
    assert len(k_sched) == TILES
    kpref = np.concatenate([[0], np.cumsum([k + 1 for k in k_sched])])
    tot_slots = int(kpref[-1])       # sum of (K_t + 1) over tiles
    kg_max = max(k_sched) + 1

    # ---- I/O ----
    hT = nc.dram_tensor("ht", [2 * 128, N_PAD], F16, kind="ExternalInput")
    wa = nc.dram_tensor("wa", [2 * 128, DOUT], F16, kind="ExternalInput")
    ident = nc.dram_tensor("ident", [128, 128], F16, kind="ExternalInput")
    unrot = nc.dram_tensor("unrot", [2 * 128, DOUT], F16, kind="ExternalInput")
    crep_d = nc.dram_tensor("crep", [128, 2 * H], F16, kind="ExternalInput")
    idx_d = nc.dram_tensor("idx", [128, tot_slots * 8], I16, kind="ExternalInput")
    sent_d = nc.dram_tensor("sent", [128, ROWE], F16, kind="ExternalInput")
    out_d = nc.dram_tensor("out", [SHARD, DOUT], F32, kind="ExternalOutput")

    with tile.TileContext(nc) as tc:
        import contextlib

        ctx = contextlib.ExitStack()
        with ctx:
            consts = ctx.enter_context(tc.tile_pool(name="consts", bufs=1))
            dram = ctx.enter_context(tc.tile_pool(name="dram", bufs=1, space="DRAM"))

            table = dram.tile([N_PAD + 128, ROWE], F16)

            # constants in
            wa0 = consts.tile([128, DOUT], F16)
            wa1 = consts.tile([128, DOUT], F16)
            nc.sync.dma_start(out=wa0[:], in_=wa[0:128, :])
            nc.sync.dma_start(out=wa1[:], in_=wa[128:256, :])
            idn = consts.tile([128, 128], F16)
            nc.sync.dma_start(out=idn[:], in_=ident[:, :])
            # un-rotation matrix chunks + el-combination coefficients
            u0 = consts.tile([128, DOUT], F16)
            u1 = consts.tile([128, DOUT], F16)
            nc.sync.dma_start(out=u0[:], in_=unrot[0:128, :])
            nc.sync.dma_start(out=u1[:], in_=unrot[128:256, :])
            crep = consts.tile([128, 2 * H], F16)
            nc.sync.dma_start(out=crep[:], in_=crep_d[:, :])
            idx_sb = consts.tile([128, tot_slots * 8], I16)
            nc.sync.dma_start(out=idx_sb[:], in_=idx_d[:, :])
            # sentinel rows: features chosen so er_h = -150 for every head;
            # masked/padded slots gather them and die in the softmax exp.
            sent_sb = consts.tile([128, ROWE], F16)
            nc.sync.dma_start(out=sent_sb[:], in_=sent_d[:, :])
            nc.sync.dma_start(out=table[N_PAD:N_PAD + 128, :], in_=sent_sb[:])

            nc.gpsimd.load_library(library_config.mlp)

            # ---------------- phase 1: projection + table build ----------------
            with (
                tc.tile_pool(name="strips", bufs=2) as strips,
                tc.tile_pool(name="p1psum", bufs=6, space="PSUM") as p1psum,
                tc.tile_pool(name="tab", bufs=4) as tabp,
            ):
                blk_per_strip = n_strip // 128
                for s in range(N_PAD // n_strip):
                    st0 = strips.tile([128, n_strip], F16, tag="st0")
                    st1 = strips.tile([128, n_strip], F16, tag="st1")
                    c0 = s * n_strip
                    nc.sync.dma_start(out=st0[:], in_=hT[0:128, c0:c0 + n_strip])
                    nc.sync.dma_start(out=st1[:], in_=hT[128:256, c0:c0 + n_strip])
                    for b in range(blk_per_strip):
                        g = s * blk_per_strip + b
                        ps = p1psum.tile([128, DOUT], F32)
                        nc.tensor.matmul(
                            out=ps[:],
                            lhsT=st0[:, b * 128:(b + 1) * 128],
                            rhs=wa0[:],
                            start=True, stop=False,
                        )
                        nc.tensor.matmul(
                            out=ps[:],
                            lhsT=st1[:, b * 128:(b + 1) * 128],
                            rhs=wa1[:],
                            start=False, stop=True,
                        )
                        tb = tabp.tile([128, ROWE], F16)
                        nc.scalar.copy(tb[:], ps[:])
                        nc.sync.dma_start(
                            out=table[g * 128:(g + 1) * 128, :], in_=tb[:]
                        )

            # ---------------- phase 2: gather / attention ----------------
            pre = consts.tile([128, TILES, DOUT], F32)   # pre-activation rows
            gbuf = consts.tile([128, TILES, DOUT], F32)
            vinv = consts.tile([128, TILES], F32)
            rstd = consts.tile([128, TILES], F32)
            mus = consts.tile([128, TILES], F32)

            def epilogue(ln, t0, t1):
                # gelu + LayerNorm (gamma=1, beta=0 per spec fills) + out DMA
                nc.scalar.activation(
                    gbuf[:, t0:t1, :].rearrange("p t f -> p (t f)"),
                    pre[:, t0:t1, :].rearrange("p t f -> p (t f)"),
                    AF.Gelu,
                )
                for t in range(t0, t1):
                    stats = ln.tile([128, 6], F32, tag="st")
                    nc.vector.bn_stats(out=stats[:], in_=gbuf[:, t, :])
                    mv = ln.tile([128, 2], F32, tag="mv")
                    nc.vector.bn_aggr(out=mv[:], in_=stats[:])
                    nc.vector.tensor_copy(mus[:, t:t + 1], mv[:, 0:1])
                    veps = ln.tile([128, 1], F32, tag="veps")
                    nc.vector.tensor_scalar_add(veps[:], mv[:, 1:2], LN_EPS)
                    nc.vector.reciprocal(vinv[:, t:t + 1], veps[:])
                nc.scalar.sqrt(rstd[:, t0:t1], vinv[:, t0:t1])
                for t in range(t0, t1):
                    nc.vector.scalar_tensor_tensor(
                        out=pre[:, t, :],
                        in0=gbuf[:, t, :],
                        scalar=mus[:, t:t + 1],
                        in1=rstd[:, t:t + 1].to_broadcast([128, DOUT]),
                        op0=ALU.subtract, op1=ALU.mult,
                    )
                nc.sync.dma_start(
                    out=out_d[t0 * 128:t1 * 128, :].rearrange(
                        "(t p) f -> p t f", p=128
                    ),
                    in_=pre[:, t0:t1, :],
                )

            with (
                tc.tile_pool(name="gat", bufs=3) as gat,
                tc.tile_pool(name="sc", bufs=3) as sc,
                tc.tile_pool(name="prod", bufs=2) as prodp,
                tc.tile_pool(name="ln", bufs=4) as lnp,
                tc.tile_pool(name="pT", bufs=2, space="PSUM") as pTp,
                tc.tile_pool(name="p2psum", bufs=2, space="PSUM") as p2psum,
            ):
                for t in range(TILES):
                    if t == TILES - 2:
                        epilogue(lnp, 0, TILES - 2)
                    K = k_sched[t]
                    KG = K + 1             # + self slot
                    s0 = int(kpref[t])     # slot offset into idx/mask arrays
                    G = gat.tile([128, kg_max, ROWE], F16, tag="G")
                    for c0 in range(0, KG, KCH):
                        c1 = min(c0 + KCH, KG)
                        ni = (c1 - c0) * 128
                        nc.gpsimd.dma_gather(
                            G[:, c0:c1, :],
                            table[:, :],
                            idx_sb[:, (s0 + c0) * 8:(s0 + c1) * 8],
                            ni,
                            ni,
                            ROWE,
                            elem_step=ROWE,
                        )

                    # rotated basis: er = feature 0 of each head block;
                    # el = c0*f0 + c1*f1 of the self row
                    elp = sc.tile([128, H, 2], F32, tag="elp")
                    selfv = G[:, K, :].rearrange("p (j d) -> p j d", d=D)[:, :, 0:2]
                    nc.vector.tensor_mul(
                        elp[:], selfv,
                        crep[:].rearrange("p (j c) -> p j c", c=2),
                    )
                    el = sc.tile([128, H], F32, tag="el")
                    nc.vector.tensor_reduce(
                        out=el[:], in_=elp[:], axis=AX.X, op=ALU.add
                    )
                    er_v = (
                        G[:, 0:K, :]
                        .rearrange("p k (j d) -> p j k d", d=D)[:, :, :, 0:1]
                        .rearrange("p j k o -> p j (k o)")
                    )
                    S = sc.tile([128, H, K], F32, tag="S")
                    nc.vector.tensor_add(
                        S[:], er_v, el[:, :, None].to_broadcast([128, H, K])
                    )
                    # leaky relu: (S*0.2) max S, then + maskln
                    S2 = sc.tile([128, H, K], F32, tag="S2")
                    nc.vector.scalar_tensor_tensor(
                        out=S2[:], in0=S[:], scalar=0.2, in1=S[:],
                        op0=ALU.mult, op1=ALU.max,
                    )
                    E = sc.tile([128, H, K], F32, tag="E")
                    nc.scalar.activation(E[:], S2[:], AF.Exp)
                    dsum = sc.tile([128, H], F32, tag="dsum")
                    nc.vector.tensor_reduce(
                        out=dsum[:], in_=E[:], axis=AX.X, op=ALU.add
                    )
                    rinv = sc.tile([128, H], F32, tag="rinv")
                    nc.vector.reciprocal(rinv[:], dsum[:])
                    alph = sc.tile([128, H, K], F16, tag="alph")
                    nc.vector.tensor_mul(
                        alph[:], E[:], rinv[:, :, None].to_broadcast([128, H, K])
                    )
                    # weighted neighbor features: alpha broadcast over d
                    prod = prodp.tile([128, kg_max, DOUT], F16, tag="prod")
                    al_v = alph[:].rearrange("p j k -> p k j")[:, :, :, None]
                    nc.vector.tensor_mul(
                        prod[:, 0:K, :],
                        G[:, 0:K, :].rearrange("p k (j d) -> p k j d", d=D),
                        al_v.to_broadcast([128, K, H, D]),
                    )
                    # transposed slot-sum on PE (psum[f', n]), then un-rotate
                    sbT0 = sc.tile([128, 128], F16, tag="sbT0")
                    sbT1 = sc.tile([128, 128], F16, tag="sbT1")
                    for c, sbT in ((0, sbT0), (1, sbT1)):
                        poT = pTp.tile([128, 128], F32, tag=f"poT{c}")
                        nc.tensor.matmul(
                            out=poT[:], lhsT=G[:, K, c * 128:(c + 1) * 128],
                            rhs=idn[:], start=True, stop=False,
                        )
                        for j in range(K):
                            nc.tensor.matmul(
                                out=poT[:],
                                lhsT=prod[:, j, c * 128:(c + 1) * 128],
                                rhs=idn[:], start=False, stop=(j == K - 1),
                            )
                        nc.vector.tensor_copy(sbT[:], poT[:])
                    p2t = p2psum.tile([128, DOUT], F32)
                    nc.tensor.matmul(
                        out=p2t[:], lhsT=sbT0[:], rhs=u0[:],
                        start=True, stop=False,
                    )
                    nc.tensor.matmul(
                        out=p2t[:], lhsT=sbT1[:], rhs=u1[:],
                        start=False, stop=True,
                    )
                    nc.vector.tensor_copy(pre[:, t, :], p2t[:])

                epilogue(lnp, TILES - 2, TILES)
    return nc


def build_nc(k_sched):
    nc = bacc.Bacc("TRN2", target_bir_lowering=False, debug=False)
    build_graph(nc, k_sched)
    nc.compile()
    return nc


# ---------------------------------------------------------------------------
# host-side planning + marshaling (pure layout/dtype work)
# ---------------------------------------------------------------------------

def plan(neighbor_mask):
    """Sort nodes by unmasked-degree into 160 tiles; deal tiles round-robin
    to cores; derive the shared per-round slot schedule."""
    cnt = np.zeros(N_PAD, np.int64)
    cnt[:N] = (neighbor_mask != 0).sum(1)
    order = np.argsort(-cnt, kind="stable")         # descending degree
    gtiles = order.reshape(GTILES, 128)             # global tile g, partition p
    tile_max = cnt[gtiles].max(1)                   # per-tile max degree
    k_sched = tuple(
        int(max(1, tile_max[8 * r:8 * r + 8].max())) for r in range(TILES)
    )
    rank = np.empty(N_PAD, np.int64)
    rank[order] = np.arange(N_PAD)                  # node -> sorted position
    return order, gtiles, rank, k_sched


def make_inputs(h, neighbor_idx, neighbor_mask, W, a_l, a_r,
                order, gtiles, rank, k_sched):
    kpref = np.concatenate([[0], np.cumsum([k + 1 for k in k_sched])])
    tot_slots = int(kpref[-1])

    # h columns in sorted order; pad nodes (id >= N) stay zero
    h16 = h.astype(np.float16)
    hT = np.zeros((2 * 128, N_PAD), np.float16)
    real_cols = np.where(order < N)[0]
    hT[:, real_cols] = h16[order[real_cols]].T

    ident = np.eye(128, dtype=np.float16)

    # per-head rotation: component 0 of each rotated block IS er, and
    # el = c0*f0 + c1*f1; un-rotation U maps the slot-sum back.
    Wr = np.zeros((DIN, DOUT), np.float64)
    Ufull = np.zeros((DOUT, DOUT), np.float64)
    cvec = np.zeros(2 * H, np.float64)
    W64 = W.astype(np.float64)
    for hh in range(H):
        ar = a_r[hh].astype(np.float64)
        al = a_l[hh].astype(np.float64)
        d0 = np.linalg.norm(ar)
        r0 = ar / d0
        v = al - (al @ r0) * r0
        nv = np.linalg.norm(v)
        if nv < 1e-9:
            v = np.zeros(D)
            v[int(np.argmin(np.abs(r0)))] = 1.0
            v -= (v @ r0) * r0
            nv = np.linalg.norm(v)
        r1 = v / nv
        q, _ = np.linalg.qr(np.column_stack([r0, r1, np.eye(D)]))
        R = q[:, :D].T
        if R[0] @ r0 < 0:
            R[0] = -R[0]
        if R[1] @ r1 < 0:
            R[1] = -R[1]
        Dv = np.ones(D)
        Dv[0] = d0
        M = R * Dv[:, None]
        U = R / Dv[:, None]
        blk = slice(hh * D, (hh + 1) * D)
        Wr[:, blk] = W64[:, blk] @ M.T
        Ufull[blk, blk] = U
        cvec[2 * hh] = (al @ r0) / d0
        cvec[2 * hh + 1] = al @ r1
    wa = np.ascontiguousarray(Wr.astype(np.float16))
    unrot = np.ascontiguousarray(Ufull.astype(np.float16))
    crep = np.ascontiguousarray(
        np.tile(cvec.astype(np.float16).reshape(1, 2 * H), (128, 1))
    )

    # sentinel features (rotated basis): er component = -150, rest 0
    sent_row = np.zeros(DOUT, np.float32)
    for hh in range(H):
        sent_row[hh * D] = -150.0
    sent = np.tile(sent_row.astype(np.float16), (128, 1))

    # per-node compacted neighbor lists in sorted-table coordinates
    nbr_rank = rank[neighbor_idx].astype(np.int16)  # [N, M]
    valid = neighbor_mask != 0
    vorder = np.argsort(~valid, axis=1, kind="stable")
    compacted = np.take_along_axis(nbr_rank, vorder, axis=1)  # valid prefix
    cnt = valid.sum(1).astype(np.int64)

    in_maps = []
    for c in range(NCORES):
        idx16 = np.full((tot_slots, 128), np.int16(N_PAD), np.int16)
        for r in range(TILES):
            K = k_sched[r]
            s0 = int(kpref[r])
            nodes = gtiles[8 * r + c]               # original node ids
            real = nodes < N
            nd = np.where(real, nodes, 0)
            sl = compacted[nd, :K].T                # [K, 128]
            have = (np.arange(K)[:, None] < cnt[nd][None, :]) & real[None, :]
            idx16[s0:s0 + K] = np.where(have, sl, np.int16(N_PAD))
            idx16[s0 + K] = ((8 * r + c) * 128
                             + np.arange(128)).astype(np.int16)  # self
        # wrap each slot-column group into 16 partitions, replicate x8
        flat = idx16.reshape(tot_slots * 128)
        wrapped = flat.reshape(tot_slots * 8, 16).T   # [16, tot*8]
        idx_in = np.ascontiguousarray(np.tile(wrapped, (8, 1)))
        in_maps.append({
            "ht": hT, "wa": wa, "ident": ident, "unrot": unrot,
            "crep": crep, "idx": idx_in, "sent": sent,
        })
    return in_maps


_CACHE = {}


def _get_nc(k_sched):
    if k_sched not in _CACHE:
        _CACHE[k_sched] = build_nc(k_sched)
    return _CACHE[k_sched]


def kernel(h, neighbor_idx, neighbor_mask, W, a_l, a_r, ln_gamma, ln_beta,
           **extra):
    assert h.shape[0] == N
    assert np.allclose(ln_gamma, 1.0) and np.allclose(ln_beta, 0.0), \
        "kernel assumes unit gamma / zero beta (per problem spec fills)"

    order, gtiles, rank, k_sched = plan(neighbor_mask)
    nc = _get_nc(k_sched)
    in_maps = make_inputs(
        h, neighbor_idx, neighbor_mask, W, a_l, a_r,
        order, gtiles, rank, k_sched,
    )
    res = run_bass_kernel_spmd(nc, in_maps, core_ids=list(range(NCORES)))
    # core c's local tile r = global tile 8r+c; sorted position = tile-major
    allout = np.stack(
        [res.results[c]["out"].reshape(TILES, 128, DOUT) for c in range(NCORES)]
    )                                              # [c, r, p, f]
    sorted_rows = allout.transpose(1, 0, 2, 3).reshape(N_PAD, DOUT)
    out = np.empty((N, DOUT), np.float32)
    sel = order < N
    out[order[sel]] = sorted_rows[sel]
    return out


# revision 18
# speedup vs baseline: 1.2068x; 1.0240x over previous
"""Trainium2 Bass kernel for nn_NodeLevelAttentionImproved (GAT-style layer).

Math (see reference):
  h_proj = h @ W                              [N, 256]
  el/er  = per-head dots of h_proj with a_l/a_r   [N, 4]
  e[n,m,h]   = leaky_relu(el[n,h] + er[idx[n,m],h], 0.2), masked -> softmax over m
  out_heads  = sum_m alpha * h_heads[idx]     [N, 4, 64]
  out = LayerNorm(gelu_erf(out_heads.flat + h_proj)) * gamma + beta

Strategy (8 cores, no collectives — each core recomputes the full projection):
  Host: nodes are sorted by unmasked-neighbor count and dealt, tile-by-tile,
  round-robin to the 8 cores, so all cores share one per-round gather-slot
  schedule K_sched (identical NEFF on every core) while skipping masked
  neighbor slots entirely (~2x fewer gather rows than the dense M=32 layout).

  phase 1: full h_proj via PE (fp16), streamed to a DRAM table of bare
           512-byte feature rows (sorted node order).
  phase 2: per output tile of 128 nodes, chunked dma_gather of K_t
           unmasked-neighbor rows + the self row. The per-head feature
           blocks are pre-rotated (host-side orthogonal basis) so that
           component 0 IS er and el = c0*f0 + c1*f1 of the self row -- no
           DVE dot products. Masked/padded slots gather a sentinel row
           whose er component is -150 (dies in the softmax exp; no mask
           tensor, no max subtraction; exp in fp32). Alpha is applied by
           DVE broadcast-multiply; the slot-sum runs transposed on the PE
           (psum[f', n], residual folded in) followed by an un-rotation
           matmul back to psum[n, f]. Gelu + LayerNorm run in two batches
           (first one overlapped under the remaining gathers).
"""

import sys

for _p in ("/opt/trn_rl_repo", "/root/.axon_site/_ro/trn_rl_repo"):
    if _p not in sys.path:
        sys.path.insert(0, _p)

import numpy as np

import concourse.bacc as bacc
import concourse.bass as bass
import concourse.mybir as mybir
import concourse.tile as tile
from concourse import library_config
from concourse.bass_utils import run_bass_kernel_spmd

F32 = mybir.dt.float32
F16 = mybir.dt.float16
I16 = mybir.dt.int16
AF = mybir.ActivationFunctionType
ALU = mybir.AluOpType
AX = mybir.AxisListType

# Problem constants (hardcoded per the harness contract).
N = 20000
M = 32          # neighbors
DIN = 256
DOUT = 256
H = 4
D = 64
LN_EPS = 1e-5
NCORES = 8
N_PAD = 20480
SHARD = N_PAD // NCORES
TILES = SHARD // 128          # 20 tiles of 128 nodes per core
GTILES = N_PAD // 128         # 160 global tiles

ROWE = 256       # fp16 elements per table row (512B)
KBLK = 2048      # h_T strip width for phase-1 loads
KCH = 7          # gather slots per chunk (7*128 = 896 descriptors)


def build_graph(nc, k_sched, n_strip=KBLK):
    """Emit the full per-core program into `nc` (inside a TileContext).

    k_sched: per-round gather-slot counts (neighbors only; the self row is
    always appended as slot K), shared by all cores.
    """
    assert len(k_sched) == TILES
    kpref = np.concatenate([[0], np.cumsum([k + 1 for k in k_sched])])
    tot_slots = int(kpref[-1])       # sum of (K_t + 1) over tiles
    kg_max = max(k_sched) + 1

    # ---- I/O ----
    hT = nc.dram_tensor("ht", [2 * 128, N_PAD], F16, kind="ExternalInput")
    wa = nc.dram_tensor("wa", [2 * 128, DOUT], F16, kind="ExternalInput")
    ident = nc.dram_tensor("ident", [128, 128], F16, kind="ExternalInput")
    unrot = nc.dram_tensor("unrot", [2 * 128, DOUT], F16, kind="ExternalInput")
    crep_d = nc.dram_tensor("crep", [128, 2 * H], F16, kind="ExternalInput")
    idx_d = nc.dram_tensor("idx", [128, tot_slots * 8], I16, kind="ExternalInput")
    sent_d = nc.dram_tensor("sent", [128, ROWE], F16, kind="ExternalInput")
    out_d = nc.dram_tensor("out", [SHARD, DOUT], F32, kind="ExternalOutput")

    with tile.TileContext(nc) as tc:
        import contextlib

        ctx = contextlib.ExitStack()
        with ctx:
            consts = ctx.enter_context(tc.tile_pool(name="consts", bufs=1))
            dram = ctx.enter_context(tc.tile_pool(name="dram", bufs=1, space="DRAM"))

            table = dram.tile([N_PAD + 128, ROWE], F16)

            # constants in
            wa0 = consts.tile([128, DOUT], F16)
            wa1 = consts.tile([128, DOUT], F16)
            nc.sync.dma_start(out=wa0[:], in_=wa[0:128, :])
            nc.sync.dma_start(out=wa1[:], in_=wa[128:256, :])
            idn = consts.tile([128, 128], F16)
            u0 = consts.tile([128, DOUT], F16)
            u1 = consts.tile([128, DOUT], F16)
            crep = consts.tile([128, 2 * H], F16)
            idx_sb = consts.tile([128, tot_slots * 8], I16)
            sent_sb = consts.tile([128, ROWE], F16)

            def load_phase2_consts():
                # deferred so the phase-1 strip loads lead the SP DMA queue
                nc.sync.dma_start(out=idn[:], in_=ident[:, :])
                nc.sync.dma_start(out=u0[:], in_=unrot[0:128, :])
                nc.sync.dma_start(out=u1[:], in_=unrot[128:256, :])
                nc.sync.dma_start(out=crep[:], in_=crep_d[:, :])
                nc.sync.dma_start(out=idx_sb[:], in_=idx_d[:, :])
                # sentinel rows: er_h = -150 for every head; masked/padded
                # slots gather them and die in the softmax exp.
                nc.sync.dma_start(out=sent_sb[:], in_=sent_d[:, :])
                nc.sync.dma_start(
                    out=table[N_PAD:N_PAD + 128, :], in_=sent_sb[:]
                )

            nc.gpsimd.load_library(library_config.mlp)

            # ---------------- phase 1: projection + table build ----------------
            with (
                tc.tile_pool(name="strips", bufs=2) as strips,
                tc.tile_pool(name="p1psum", bufs=6, space="PSUM") as p1psum,
                tc.tile_pool(name="tab", bufs=4) as tabp,
            ):
                blk_per_strip = n_strip // 128
                for s in range(N_PAD // n_strip):
                    if s == 1:
                        load_phase2_consts()
                    st0 = strips.tile([128, n_strip], F16, tag="st0")
                    st1 = strips.tile([128, n_strip], F16, tag="st1")
                    c0 = s * n_strip
                    nc.sync.dma_start(out=st0[:], in_=hT[0:128, c0:c0 + n_strip])
                    nc.sync.dma_start(out=st1[:], in_=hT[128:256, c0:c0 + n_strip])
                    for b in range(blk_per_strip):
                        g = s * blk_per_strip + b
                        ps = p1psum.tile([128, DOUT], F32)
                        nc.tensor.matmul(
                            out=ps[:],
                            lhsT=st0[:, b * 128:(b + 1) * 128],
                            rhs=wa0[:],
                            start=True, stop=False,
                        )
                        nc.tensor.matmul(
                            out=ps[:],
                            lhsT=st1[:, b * 128:(b + 1) * 128],
                            rhs=wa1[:],
                            start=False, stop=True,
                        )
                        tb = tabp.tile([128, ROWE], F16)
                        nc.scalar.copy(tb[:], ps[:])
                        nc.sync.dma_start(
                            out=table[g * 128:(g + 1) * 128, :], in_=tb[:]
                        )

            # ---------------- phase 2: gather / attention ----------------
            pre = consts.tile([128, TILES, DOUT], F32)   # pre-activation rows
            gbuf = consts.tile([128, TILES, DOUT], F32)
            vinv = consts.tile([128, TILES], F32)
            rstd = consts.tile([128, TILES], F32)
            mus = consts.tile([128, TILES], F32)

            def epilogue(ln, t0, t1):
                # gelu + LayerNorm (gamma=1, beta=0 per spec fills) + out DMA
                nc.scalar.activation(
                    gbuf[:, t0:t1, :].rearrange("p t f -> p (t f)"),
                    pre[:, t0:t1, :].rearrange("p t f -> p (t f)"),
                    AF.Gelu,
                )
                for t in range(t0, t1):
                    stats = ln.tile([128, 6], F32, tag="st")
                    nc.vector.bn_stats(out=stats[:], in_=gbuf[:, t, :])
                    mv = ln.tile([128, 2], F32, tag="mv")
                    nc.vector.bn_aggr(out=mv[:], in_=stats[:])
                    nc.vector.tensor_copy(mus[:, t:t + 1], mv[:, 0:1])
                    veps = ln.tile([128, 1], F32, tag="veps")
                    nc.vector.tensor_scalar_add(veps[:], mv[:, 1:2], LN_EPS)
                    nc.vector.reciprocal(vinv[:, t:t + 1], veps[:])
                nc.scalar.sqrt(rstd[:, t0:t1], vinv[:, t0:t1])
                for t in range(t0, t1):
                    nc.vector.scalar_tensor_tensor(
                        out=pre[:, t, :],
                        in0=gbuf[:, t, :],
                        scalar=mus[:, t:t + 1],
                        in1=rstd[:, t:t + 1].to_broadcast([128, DOUT]),
                        op0=ALU.subtract, op1=ALU.mult,
                    )
                nc.sync.dma_start(
                    out=out_d[t0 * 128:t1 * 128, :].rearrange(
                        "(t p) f -> p t f", p=128
                    ),
                    in_=pre[:, t0:t1, :],
                )

            with (
                tc.tile_pool(name="gat", bufs=3) as gat,
                tc.tile_pool(name="sc", bufs=3) as sc,
                tc.tile_pool(name="prod", bufs=2) as prodp,
                tc.tile_pool(name="ln", bufs=4) as lnp,
                tc.tile_pool(name="pT", bufs=2, space="PSUM") as pTp,
                tc.tile_pool(name="p2psum", bufs=2, space="PSUM") as p2psum,
            ):
                for t in range(TILES):
                    if t == TILES - 2:
                        epilogue(lnp, 0, TILES - 2)
                    K = k_sched[t]
                    KG = K + 1             # + self slot
                    s0 = int(kpref[t])     # slot offset into idx/mask arrays
                    G = gat.tile([128, kg_max, ROWE], F16, tag="G")
                    for c0 in range(0, KG, KCH):
                        c1 = min(c0 + KCH, KG)
                        ni = (c1 - c0) * 128
                        nc.gpsimd.dma_gather(
                            G[:, c0:c1, :],
                            table[:, :],
                            idx_sb[:, (s0 + c0) * 8:(s0 + c1) * 8],
                            ni,
                            ni,
                            ROWE,
                            elem_step=ROWE,
                        )

                    # rotated basis: er = feature 0 of each head block;
                    # el = c0*f0 + c1*f1 of the self row
                    elp = sc.tile([128, H, 2], F32, tag="elp")
                    selfv = G[:, K, :].rearrange("p (j d) -> p j d", d=D)[:, :, 0:2]
                    nc.vector.tensor_mul(
                        elp[:], selfv,
                        crep[:].rearrange("p (j c) -> p j c", c=2),
                    )
                    el = sc.tile([128, H], F32, tag="el")
                    nc.vector.tensor_reduce(
                        out=el[:], in_=elp[:], axis=AX.X, op=ALU.add
                    )
                    er_v = (
                        G[:, 0:K, :]
                        .rearrange("p k (j d) -> p j k d", d=D)[:, :, :, 0:1]
                        .rearrange("p j k o -> p j (k o)")
                    )
                    S = sc.tile([128, H, K], F32, tag="S")
                    nc.vector.tensor_add(
                        S[:], er_v, el[:, :, None].to_broadcast([128, H, K])
                    )
                    # leaky relu: (S*0.2) max S, then + maskln
                    S2 = sc.tile([128, H, K], F32, tag="S2")
                    nc.vector.scalar_tensor_tensor(
                        out=S2[:], in0=S[:], scalar=0.2, in1=S[:],
                        op0=ALU.mult, op1=ALU.max,
                    )
                    E = sc.tile([128, H, K], F32, tag="E")
                    nc.scalar.activation(E[:], S2[:], AF.Exp)
                    dsum = sc.tile([128, H], F32, tag="dsum")
                    nc.vector.tensor_reduce(
                        out=dsum[:], in_=E[:], axis=AX.X, op=ALU.add
                    )
                    rinv = sc.tile([128, H], F32, tag="rinv")
                    nc.vector.reciprocal(rinv[:], dsum[:])
                    alph = sc.tile([128, H, K], F16, tag="alph")
                    nc.vector.tensor_mul(
                        alph[:], E[:], rinv[:, :, None].to_broadcast([128, H, K])
                    )
                    # weighted neighbor features: alpha broadcast over d
                    prod = prodp.tile([128, kg_max, DOUT], F16, tag="prod")
                    al_v = alph[:].rearrange("p j k -> p k j")[:, :, :, None]
                    nc.vector.tensor_mul(
                        prod[:, 0:K, :],
                        G[:, 0:K, :].rearrange("p k (j d) -> p k j d", d=D),
                        al_v.to_broadcast([128, K, H, D]),
                    )
                    # transposed slot-sum on PE (psum[f', n]), then un-rotate
                    sbT0 = sc.tile([128, 128], F16, tag="sbT0")
                    sbT1 = sc.tile([128, 128], F16, tag="sbT1")
                    for c, sbT in ((0, sbT0), (1, sbT1)):
                        poT = pTp.tile([128, 128], F32, tag=f"poT{c}")
                        nc.tensor.matmul(
                            out=poT[:], lhsT=G[:, K, c * 128:(c + 1) * 128],
                            rhs=idn[:], start=True, stop=False,
                        )
                        for j in range(K):
                            nc.tensor.matmul(
                                out=poT[:],
                                lhsT=prod[:, j, c * 128:(c + 1) * 128],
                                rhs=idn[:], start=False, stop=(j == K - 1),
                            )
                        nc.vector.tensor_copy(sbT[:], poT[:])
                    p2t = p2psum.tile([128, DOUT], F32)
                    nc.tensor.matmul(
                        out=p2t[:], lhsT=sbT0[:], rhs=u0[:],
                        start=True, stop=False,
                    )
                    nc.tensor.matmul(
                        out=p2t[:], lhsT=sbT1[:], rhs=u1[:],
                        start=False, stop=True,
                    )
                    nc.vector.tensor_copy(pre[:, t, :], p2t[:])

                epilogue(lnp, TILES - 2, TILES)
    return nc


def build_nc(k_sched):
    nc = bacc.Bacc("TRN2", target_bir_lowering=False, debug=False)
    build_graph(nc, k_sched)
    nc.compile()
    return nc


# ---------------------------------------------------------------------------
# host-side planning + marshaling (pure layout/dtype work)
# ---------------------------------------------------------------------------

def plan(neighbor_mask):
    """Sort nodes by unmasked-degree into 160 tiles; deal tiles round-robin
    to cores; derive the shared per-round slot schedule."""
    cnt = np.zeros(N_PAD, np.int64)
    cnt[:N] = (neighbor_mask != 0).sum(1)
    order = np.argsort(-cnt, kind="stable")         # descending degree
    gtiles = order.reshape(GTILES, 128)             # global tile g, partition p
    tile_max = cnt[gtiles].max(1)                   # per-tile max degree
    k_sched = tuple(
        int(max(1, tile_max[8 * r:8 * r + 8].max())) for r in range(TILES)
    )
    rank = np.empty(N_PAD, np.int64)
    rank[order] = np.arange(N_PAD)                  # node -> sorted position
    return order, gtiles, rank, k_sched


def make_inputs(h, neighbor_idx, neighbor_mask, W, a_l, a_r,
                order, gtiles, rank, k_sched):
    kpref = np.concatenate([[0], np.cumsum([k + 1 for k in k_sched])])
    tot_slots = int(kpref[-1])

    # h columns in sorted order; pad nodes (id >= N) stay zero
    h16 = h.astype(np.float16)
    hT = np.zeros((2 * 128, N_PAD), np.float16)
    real_cols = np.where(order < N)[0]
    hT[:, real_cols] = h16[order[real_cols]].T

    ident = np.eye(128, dtype=np.float16)

    # per-head rotation: component 0 of each rotated block IS er, and
    # el = c0*f0 + c1*f1; un-rotation U maps the slot-sum back.
    Wr = np.zeros((DIN, DOUT), np.float64)
    Ufull = np.zeros((DOUT, DOUT), np.float64)
    cvec = np.zeros(2 * H, np.float64)
    W64 = W.astype(np.float64)
    for hh in range(H):
        ar = a_r[hh].astype(np.float64)
        al = a_l[hh].astype(np.float64)
        d0 = np.linalg.norm(ar)
        r0 = ar / d0
        v = al - (al @ r0) * r0
        nv = np.linalg.norm(v)
        if nv < 1e-9:
            v = np.zeros(D)
            v[int(np.argmin(np.abs(r0)))] = 1.0
            v -= (v @ r0) * r0
            nv = np.linalg.norm(v)
        r1 = v / nv
        q, _ = np.linalg.qr(np.column_stack([r0, r1, np.eye(D)]))
        R = q[:, :D].T
        if R[0] @ r0 < 0:
            R[0] = -R[0]
        if R[1] @ r1 < 0:
            R[1] = -R[1]
        Dv = np.ones(D)
        Dv[0] = d0
        M = R * Dv[:, None]
        U = R / Dv[:, None]
        blk = slice(hh * D, (hh + 1) * D)
        Wr[:, blk] = W64[:, blk] @ M.T
        Ufull[blk, blk] = U
        cvec[2 * hh] = (al @ r0) / d0
        cvec[2 * hh + 1] = al @ r1
    wa = np.ascontiguousarray(Wr.astype(np.float16))
    unrot = np.ascontiguousarray(Ufull.astype(np.float16))
    crep = np.ascontiguousarray(
        np.tile(cvec.astype(np.float16).reshape(1, 2 * H), (128, 1))
    )

    # sentinel features (rotated basis): er component = -150, rest 0
    sent_row = np.zeros(DOUT, np.float32)
    for hh in range(H):
        sent_row[hh * D] = -150.0
    sent = np.tile(sent_row.astype(np.float16), (128, 1))

    # per-node compacted neighbor lists in sorted-table coordinates
    nbr_rank = rank[neighbor_idx].astype(np.int16)  # [N, M]
    valid = neighbor_mask != 0
    vorder = np.argsort(~valid, axis=1, kind="stable")
    compacted = np.take_along_axis(nbr_rank, vorder, axis=1)  # valid prefix
    cnt = valid.sum(1).astype(np.int64)

    in_maps = []
    for c in range(NCORES):
        idx16 = np.full((tot_slots, 128), np.int16(N_PAD), np.int16)
        for r in range(TILES):
            K = k_sched[r]
            s0 = int(kpref[r])
            nodes = gtiles[8 * r + c]               # original node ids
            real = nodes < N
            nd = np.where(real, nodes, 0)
            sl = compacted[nd, :K].T                # [K, 128]
            have = (np.arange(K)[:, None] < cnt[nd][None, :]) & real[None, :]
            idx16[s0:s0 + K] = np.where(have, sl, np.int16(N_PAD))
            idx16[s0 + K] = ((8 * r + c) * 128
                             + np.arange(128)).astype(np.int16)  # self
        # wrap each slot-column group into 16 partitions, replicate x8
        flat = idx16.reshape(tot_slots * 128)
        wrapped = flat.reshape(tot_slots * 8, 16).T   # [16, tot*8]
        idx_in = np.ascontiguousarray(np.tile(wrapped, (8, 1)))
        in_maps.append({
            "ht": hT, "wa": wa, "ident": ident, "unrot": unrot,
            "crep": crep, "idx": idx_in, "sent": sent,
        })
    return in_maps


_CACHE = {}


def _get_nc(k_sched):
    if k_sched not in _CACHE:
        _CACHE[k_sched] = build_nc(k_sched)
    return _CACHE[k_sched]


def kernel(h, neighbor_idx, neighbor_mask, W, a_l, a_r, ln_gamma, ln_beta,
           **extra):
    assert h.shape[0] == N
    assert np.allclose(ln_gamma, 1.0) and np.allclose(ln_beta, 0.0), \
        "kernel assumes unit gamma / zero beta (per problem spec fills)"

    order, gtiles, rank, k_sched = plan(neighbor_mask)
    nc = _get_nc(k_sched)
    in_maps = make_inputs(
        h, neighbor_idx, neighbor_mask, W, a_l, a_r,
        order, gtiles, rank, k_sched,
    )
    res = run_bass_kernel_spmd(nc, in_maps, core_ids=list(range(NCORES)))
    # core c's local tile r = global tile 8r+c; sorted position = tile-major
    allout = np.stack(
        [res.results[c]["out"].reshape(TILES, 128, DOUT) for c in range(NCORES)]
    )                                              # [c, r, p, f]
    sorted_rows = allout.transpose(1, 0, 2, 3).reshape(N_PAD, DOUT)
    out = np.empty((N, DOUT), np.float32)
    sel = order < N
    out[order[sel]] = sorted_rows[sel]
    return out
